# revision 10
# baseline (speedup 1.0000x reference)
"""Trainium2 Bass kernel for nn_CharTaggerBiLSTM.

Data-parallel over batch B=128 across 8 NeuronCores (16 sentences/core).
Per core, fully on-device:
  1. Char LSTM (20 steps) over 2048 words in transposed layout
     (features-on-partitions), f32r matmuls, masked last-state capture.
  2. Word BiLSTM (128 steps x 2 dirs) in rows-layout gates with
     weights-as-moving-operand matmuls; per-step PE transposes feed the
     transposed hidden buffer consumed by both the recurrence and the MLP.
  3. MLP (1024->256->256->50) + log_softmax, output [2048, 50] per core.

Host side only does: embedding gather for the char inputs, weight
transposes/reshapes, and final reassembly.
"""

import sys
import functools
from contextlib import ExitStack

sys.path.insert(0, "/opt/trn_rl_repo")

import numpy as np
from concourse import bacc, bass, mybir, tile, bass_utils

# Problem constants (hardcoded per the harness contract).
B, S, Lc = 128, 128, 20
AB, E = 100, 64
Hc, H, OUT = 256, 512, 50
NCORE = 8
BL = B // NCORE          # sentences per core
FP = mybir.dt.float32
FR = mybir.dt.float32r
G4 = 4 * Hc              # char gate width (1024)
WG = 4 * H               # word gate width (2048)


def _ceil_div(a, b):
    return (a + b - 1) // b


def build_module(bl=BL):
    """Build + compile the per-core Bass module. bl = sentences per core."""
    nl = bl * S  # char rows / word positions per core
    nc = bacc.Bacc("TRN2", target_bir_lowering=False, debug=False,
                   num_devices=NCORE)

    # ---- DRAM I/O (per core) ----
    d_eT = nc.dram_tensor("eT", [Lc, E, nl], FR, kind="ExternalInput")
    d_lenrep = nc.dram_tensor("lenrep", [128, nl], FP, kind="ExternalInput")
    d_cWxT = nc.dram_tensor("cWxT", [E, G4], FR, kind="ExternalInput")
    d_cWhT = nc.dram_tensor("cWhT", [2, 128, G4], FR, kind="ExternalInput")
    d_cbias = nc.dram_tensor("cbias", [128, G4 // 128], FP, kind="ExternalInput")
    d_wWT = nc.dram_tensor("wWT", [2, 6, 128, WG], FR, kind="ExternalInput")
    d_wbias = nc.dram_tensor("wbias", [2, 1, WG], FR, kind="ExternalInput")
    d_W1T = nc.dram_tensor("W1T", [8, 128, 256], FR, kind="ExternalInput")
    d_b1 = nc.dram_tensor("b1m", [128, 2], FP, kind="ExternalInput")
    d_W2T = nc.dram_tensor("W2T", [2, 128, 256], FR, kind="ExternalInput")
    d_b2 = nc.dram_tensor("b2m", [128, 2], FP, kind="ExternalInput")
    d_W3T = nc.dram_tensor("W3T", [2, 128, OUT], FR, kind="ExternalInput")
    d_b3 = nc.dram_tensor("b3m", [OUT, 1], FP, kind="ExternalInput")
    d_eye = nc.dram_tensor("eye", [128, 128], FP, kind="ExternalInput")
    d_ones = nc.dram_tensor("onesr", [1, 128], FR, kind="ExternalInput")
    d_y = nc.dram_tensor("y", [nl, OUT], FP, kind="ExternalOutput")

    CH = min(512, nl)            # char-phase column chunk
    NCH = _ceil_div(nl, CH)
    Sig = mybir.ActivationFunctionType.Sigmoid
    Tanh = mybir.ActivationFunctionType.Tanh
    Relu = mybir.ActivationFunctionType.Relu
    Exp = mybir.ActivationFunctionType.Exp
    Ln = mybir.ActivationFunctionType.Ln
    Ident = mybir.ActivationFunctionType.Identity

    with TileCtx(nc) as tc:
        with ExitStack() as ctx:
            # Persistent across the whole kernel.
            persist = ctx.enter_context(tc.tile_pool(name="persist", bufs=1))
            eye_sb = persist.tile([128, 128], FP, tag="eye", name="eye")
            nc.sync.dma_start(eye_sb[:], d_eye.ap()[:])
            # word-BiLSTM hidden outputs, transposed: tiles 0-3 fwd, 4-7 bwd
            hsT = [persist.tile([128, nl], FR, tag=f"hsT{j}", name=f"hsT{j}") for j in range(8)]

            c12 = ctx.enter_context(ExitStack())
            midp = c12.enter_context(tc.tile_pool(name="mid", bufs=1))
            # char-LSTM final states, transposed [Hc, nl] as 2 tiles
            last = [midp.tile([128, nl], FR, tag=f"last{j}", name=f"last{j}") for j in range(2)]

            # ================= Phase 1: char LSTM =================
            with ExitStack() as c1:
                cw = c1.enter_context(tc.tile_pool(name="cweights", bufs=1))
                cst = c1.enter_context(tc.tile_pool(name="cstate", bufs=1))
                ein = c1.enter_context(tc.tile_pool(name="ein", bufs=2))
                ctmp = c1.enter_context(tc.tile_pool(name="ctmp", bufs=2))
                cps = c1.enter_context(
                    tc.tile_pool(name="cpsum", bufs=8, space="PSUM"))

                cWx = cw.tile([E, G4], FR, tag="cWx", name="cWx")
                cWh = cw.tile([128, 2, G4], FR, tag="cWh", name="cWh")
                cb = cw.tile([128, G4 // 128], FP, tag="cb", name="cb")
                lenr = cw.tile([128, nl], FP, tag="lenr", name="lenr")
                nc.sync.dma_start(cWx[:], d_cWxT.ap()[:])
                nc.sync.dma_start(cWh[:], d_cWhT.ap().rearrange("k p g -> p k g"))
                nc.sync.dma_start(cb[:], d_cbias.ap()[:])
                nc.sync.dma_start(lenr[:], d_lenrep.ap()[:])
                for j in range(2):
                    nc.vector.tensor_scalar_mul(last[j][:], lenr[:, :], 0.0)

                # h + c state, transposed layout (h updated chunk-in-place)
                hh = [cst.tile([128, nl], FR, tag=f"h{j}", name=f"h{j}") for j in range(2)]
                cc = [cst.tile([128, nl], FP, tag=f"c{j}", name=f"c{j}") for j in range(2)]
                for j in range(2):
                    nc.vector.memset(cc[j][:], 0.0)

                for t in range(Lc):
                    et = ein.tile([E, nl], FR, tag="et", name="et")
                    nc.sync.dma_start(et[:], d_eT.ap()[t])
                    hprev = hcur = hh
                    for ci in range(NCH):
                        cs = slice(ci * CH, (ci + 1) * CH)
                        mask = ctmp.tile([128, CH], FP, tag="mask", name="mask")
                        nc.vector.tensor_scalar(
                            mask[:], lenr[:, cs], float(t), None,
                            op0=mybir.AluOpType.is_gt)
                        # all 8 gate m-tiles' matmuls first (they read hprev),
                        # then the elementwise updates (which overwrite h)
                        ps = {}
                        for j in range(2):
                            for gi in range(4):
                                m = 2 * gi + j
                                p = cps.tile([128, CH], FP, tag="ps", name="ps")
                                ps[(j, gi)] = p
                                mm = []
                                mm.append((cWx[:, m * 128:(m + 1) * 128], et[:, cs]))
                                if t > 0:
                                    for k in range(2):
                                        mm.append((cWh[:, k, m * 128:(m + 1) * 128],
                                                   hprev[k][:, cs]))
                                for ki, (lhsT, rhs) in enumerate(mm):
                                    nc.tensor.matmul(
                                        p[:], lhsT, rhs,
                                        start=(ki == 0), stop=(ki == len(mm) - 1))
                        for j in range(2):
                            bias = [cb[:, (2 * gi + j):(2 * gi + j) + 1]
                                    for gi in range(4)]
                            i_s = ctmp.tile([128, CH], FP, tag="i_s", name="i_s")
                            f_s = ctmp.tile([128, CH], FP, tag="f_s", name="f_s")
                            g_t = ctmp.tile([128, CH], FP, tag="g_t", name="g_t")
                            o_s = ctmp.tile([128, CH], FP, tag="o_s", name="o_s")
                            nc.scalar.activation(i_s[:], ps[(j, 0)][:], Sig, bias=bias[0])
                            nc.scalar.activation(f_s[:], ps[(j, 1)][:], Sig, bias=bias[1])
                            nc.scalar.activation(g_t[:], ps[(j, 2)][:], Tanh, bias=bias[2])
                            nc.scalar.activation(o_s[:], ps[(j, 3)][:], Sig, bias=bias[3])
                            ig = ctmp.tile([128, CH], FP, tag="ig", name="ig")
                            nc.vector.tensor_mul(ig[:], i_s[:], g_t[:])
                            nc.vector.tensor_mul(cc[j][:, cs], f_s[:], cc[j][:, cs])
                            nc.vector.tensor_add(cc[j][:, cs], cc[j][:, cs], ig[:])
                            tc_t = ctmp.tile([128, CH], FP, tag="tc", name="tc")
                            nc.scalar.activation(tc_t[:], cc[j][:, cs], Tanh)
                            nc.vector.tensor_mul(hcur[j][:, cs], o_s[:], tc_t[:])
                            d = ctmp.tile([128, CH], FP, tag="d", name="d")
                            nc.vector.tensor_sub(d[:], hcur[j][:, cs],
                                                 last[j][:, cs])
                            nc.vector.tensor_mul(d[:], d[:], mask[:])
                            nc.vector.tensor_add(last[j][:, cs],
                                                 last[j][:, cs], d[:])

            # ================= Phase 2: word BiLSTM =================
            with ExitStack() as c2:
                ww = c2.enter_context(tc.tile_pool(name="wweights", bufs=1))
                wst = c2.enter_context(tc.tile_pool(name="wstate", bufs=1))
                wtmp = c2.enter_context(tc.tile_pool(name="wtmp", bufs=2))
                wps = c2.enter_context(
                    tc.tile_pool(name="wpsum", bufs=6, space="PSUM"))
                tps = c2.enter_context(
                    tc.tile_pool(name="tpsum", bufs=2, space="PSUM"))

                ones = ww.tile([1, bl], FR, tag="ones", name="ones")
                nc.sync.dma_start(ones[:], d_ones.ap()[:, 0:bl])
                wb = [ww.tile([1, WG], FR, tag=f"wb{d}", name=f"wb{d}") for d in range(2)]
                for d in range(2):
                    nc.sync.dma_start(wb[d][:], d_wbias.ap()[d])
                cstate = [wst.tile([bl, H], FP, tag=f"wc{d}", name=f"wc{d}") for d in range(2)]

                # strided views: column block for word-step s
                lastv = [last[j].rearrange("p (b s) -> p s b", s=S)
                         for j in range(2)]
                hsTv = [hsT[j].rearrange("p (b s) -> p s b", s=S)
                        for j in range(8)]

                for d in range(2):
                    wsb = ww.tile([128, 6, WG], FR, tag="wsb", name="wsb")
                    nc.sync.dma_start(
                        wsb[:], d_wWT.ap()[d].rearrange("k p g -> p k g"))
                    nc.vector.memset(cstate[d][:], 0.0)
                    for si in range(S):
                        s = si if d == 0 else S - 1 - si
                        sprev = s - 1 if d == 0 else s + 1
                        # gates [bl, WG] in 4 chunks (i,f,g,o), psum
                        gps = []
                        for gc in range(4):
                            p = wps.tile([bl, H], FP, tag="wps", name="wps")
                            gps.append(p)
                            ns = slice(gc * H, (gc + 1) * H)
                            # bias row via K=1 ones matmul
                            nc.tensor.matmul(p[:], ones[:],
                                             wb[d][:, ns],
                                             start=True, stop=False)
                            # char-input part: K = Hc as 2 tiles
                            for k in range(2):
                                nc.tensor.matmul(
                                    p[:], lastv[k][:, s, :],
                                    wsb[:, k, ns],
                                    start=False, stop=(si == 0 and k == 1))
                            # recurrent part: K = H as 4 tiles
                            if si > 0:
                                for k in range(4):
                                    nc.tensor.matmul(
                                        p[:],
                                        hsTv[4 * d + k][:, sprev, :],
                                        wsb[:, 2 + k, ns],
                                        start=False, stop=(k == 3))
                        i_s = wtmp.tile([bl, H], FP, tag="wi", name="wi")
                        f_s = wtmp.tile([bl, H], FP, tag="wf", name="wf")
                        g_t = wtmp.tile([bl, H], FP, tag="wg", name="wg")
                        o_s = wtmp.tile([bl, H], FP, tag="wo", name="wo")
                        nc.scalar.activation(i_s[:], gps[0][:], Sig)
                        nc.scalar.activation(f_s[:], gps[1][:], Sig)
                        nc.scalar.activation(g_t[:], gps[2][:], Tanh)
                        nc.scalar.activation(o_s[:], gps[3][:], Sig)
                        ig = wtmp.tile([bl, H], FP, tag="wig", name="wig")
                        nc.vector.tensor_mul(ig[:], i_s[:], g_t[:])
                        nc.vector.tensor_mul(cstate[d][:], f_s[:], cstate[d][:])
                        nc.vector.tensor_add(cstate[d][:], cstate[d][:], ig[:])
                        tc_t = wtmp.tile([bl, H], FP, tag="wtc", name="wtc")
                        nc.scalar.activation(tc_t[:], cstate[d][:], Tanh)
                        hrow = wtmp.tile([bl, H], FP, tag="whr", name="whr")
                        nc.vector.tensor_mul(hrow[:], o_s[:], tc_t[:])
                        # transpose h [bl, H] -> 4 x [128, bl] into hsT
                        for k in range(4):
                            tp = tps.tile([128, bl], FP, tag="tp", name="tp")
                            nc.tensor.transpose(
                                tp[:], hrow[:, k * 128:(k + 1) * 128],
                                eye_sb[0:bl, 0:bl])
                            nc.vector.tensor_copy(hsTv[4 * d + k][:, s, :], tp[:])

            c12.close()

            # ================= Phase 3: MLP + log_softmax =================
            with ExitStack() as c3:
                mw = c3.enter_context(tc.tile_pool(name="mweights", bufs=1))
                mact = c3.enter_context(tc.tile_pool(name="mact", bufs=1))
                mtmp = c3.enter_context(tc.tile_pool(name="mtmp", bufs=4))
                mps = c3.enter_context(
                    tc.tile_pool(name="mpsum", bufs=2, space="PSUM"))
                sps = c3.enter_context(
                    tc.tile_pool(name="spsum", bufs=2, space="PSUM"))

                W1 = mw.tile([128, 8, 256], FR, tag="W1", name="W1")
                W2 = mw.tile([128, 2, 256], FR, tag="W2", name="W2")
                W3 = mw.tile([128, 2, OUT], FR, tag="W3", name="W3")
                b1 = mw.tile([128, 2], FP, tag="b1", name="b1")
                b2 = mw.tile([128, 2], FP, tag="b2", name="b2")
                b3 = mw.tile([OUT, 1], FP, tag="b3", name="b3")
                nc.sync.dma_start(W1[:], d_W1T.ap().rearrange("k p g -> p k g"))
                nc.sync.dma_start(W2[:], d_W2T.ap().rearrange("k p g -> p k g"))
                nc.sync.dma_start(W3[:], d_W3T.ap().rearrange("k p g -> p k g"))
                nc.sync.dma_start(b1[:], d_b1.ap()[:])
                nc.sync.dma_start(b2[:], d_b2.ap()[:])
                nc.sync.dma_start(b3[:], d_b3.ap()[:])

                h1 = [mact.tile([128, nl], FR, tag=f"h1{m}", name=f"h1{m}") for m in range(2)]
                h2 = [mact.tile([128, nl], FR, tag=f"h2{m}", name=f"h2{m}") for m in range(2)]

                for ci in range(NCH):
                    cs = slice(ci * CH, (ci + 1) * CH)
                    for m in range(2):
                        p = mps.tile([128, CH], FP, tag="mp1", name="mp1")
                        for k in range(8):
                            nc.tensor.matmul(
                                p[:], W1[:, k, m * 128:(m + 1) * 128],
                                hsT[k][:, cs],
                                start=(k == 0), stop=(k == 7))
                        nc.scalar.activation(h1[m][:, cs], p[:], Relu,
                                             bias=b1[:, m:m + 1])
                for ci in range(NCH):
                    cs = slice(ci * CH, (ci + 1) * CH)
                    for m in range(2):
                        p = mps.tile([128, CH], FP, tag="mp2", name="mp2")
                        for k in range(2):
                            nc.tensor.matmul(
                                p[:], W2[:, k, m * 128:(m + 1) * 128],
                                h1[k][:, cs],
                                start=(k == 0), stop=(k == 1))
                        nc.scalar.activation(h2[m][:, cs], p[:], Relu,
                                             bias=b2[:, m:m + 1])
                # logits + log_softmax, per 128-position tile
                for pi in range(nl // 128 if nl >= 128 else 1):
                    pcount = min(128, nl - pi * 128)
                    psl = slice(pi * 128, pi * 128 + pcount)
                    lg = mps.tile([OUT, pcount], FP, tag="mp3", name="mp3")
                    for k in range(2):
                        nc.tensor.matmul(
                            lg[:], W3[:, k, :],
                            h2[k][:, psl],
                            start=(k == 0), stop=(k == 1))
                    lgb = mtmp.tile([OUT, pcount], FP, tag="lgb", name="lgb")
                    nc.scalar.activation(lgb[:], lg[:], Ident, bias=b3[:, 0:1])
                    lgr = sps.tile([pcount, OUT], FP, tag="lgr", name="lgr")
                    nc.tensor.transpose(lgr[:], lgb[:], eye_sb[0:OUT, 0:OUT])
                    nmx = mtmp.tile([pcount, 1], FP, tag="nmx", name="nmx")
                    nc.vector.tensor_reduce(nmx[:], lgr[:],
                                            axis=mybir.AxisListType.X,
                                            op=mybir.AluOpType.max, negate=True)
                    ex = mtmp.tile([pcount, OUT], FP, tag="ex", name="ex")
                    sm = mtmp.tile([pcount, 1], FP, tag="sm", name="sm")
                    nc.scalar.activation(ex[:], lgr[:], Exp, bias=nmx[:],
                                         accum_out=sm[:])
                    lsm = mtmp.tile([pcount, 1], FP, tag="lsm", name="lsm")
                    nc.scalar.activation(lsm[:], sm[:], Ln)
                    shift = mtmp.tile([pcount, 1], FP, tag="shift", name="shift")
                    nc.vector.tensor_sub(shift[:], nmx[:], lsm[:])
                    yt = mtmp.tile([pcount, OUT], FP, tag="yt", name="yt")
                    nc.vector.tensor_scalar(
                        yt[:], lgr[:], shift[:], None, op0=mybir.AluOpType.add)
                    nc.sync.dma_start(d_y.ap()[psl, :], yt[:])

    nc.compile()
    return nc


def TileCtx(nc):
    return tile.TileContext(nc)


@functools.lru_cache(maxsize=2)
def _cached_module(bl):
    return build_module(bl)


def _host_prep(inputs, bl):
    """Build the 8 per-core in_maps from the full inputs."""
    x = np.asarray(inputs["x"])
    emb = np.asarray(inputs["emb"], dtype=np.float32)
    nl = bl * S
    f32 = np.float32

    cWxT = np.ascontiguousarray(np.asarray(inputs["cW_ih"], f32).T)
    cWhT = np.ascontiguousarray(
        np.asarray(inputs["cW_hh"], f32).T).reshape(2, 128, G4)
    cbias = (np.asarray(inputs["cb_ih"], f32) + np.asarray(inputs["cb_hh"], f32))
    cbias_m = np.ascontiguousarray(cbias.reshape(G4 // 128, 128).T)

    wWT = np.zeros((2, 6, 128, WG), f32)
    wbias = np.zeros((2, 1, WG), f32)
    for d, pre in enumerate(("f", "b")):
        wih = np.asarray(inputs[pre + "W_ih"], f32)
        whh = np.asarray(inputs[pre + "W_hh"], f32)
        wcat = np.concatenate([wih.T, whh.T], axis=0)  # [768, 2048]
        wWT[d] = wcat.reshape(6, 128, WG)
        wbias[d, 0] = (np.asarray(inputs[pre + "b_ih"], f32)
                       + np.asarray(inputs[pre + "b_hh"], f32))

    W1T = np.ascontiguousarray(
        np.asarray(inputs["W1"], f32).T).reshape(8, 128, 256)
    b1m = np.ascontiguousarray(
        np.asarray(inputs["b1"], f32).reshape(2, 128).T)
    W2T = np.ascontiguousarray(
        np.asarray(inputs["W2"], f32).T).reshape(2, 128, 256)
    b2m = np.ascontiguousarray(
        np.asarray(inputs["b2"], f32).reshape(2, 128).T)
    W3T = np.ascontiguousarray(
        np.asarray(inputs["W3"], f32).T).reshape(2, 128, OUT)
    b3m = np.ascontiguousarray(np.asarray(inputs["b3"], f32).reshape(OUT, 1))
    eye = np.eye(128, dtype=f32)

    shared = dict(cWxT=cWxT, cWhT=cWhT, cbias=cbias_m, wWT=wWT, wbias=wbias,
                  W1T=W1T, b1m=b1m, W2T=W2T, b2m=b2m, W3T=W3T, b3m=b3m,
                  eye=eye, onesr=np.ones((1, 128), f32))

    in_maps = []
    ncores = x.shape[0] // bl
    for c in range(ncores):
        xc = x[c * bl:(c + 1) * bl].reshape(nl, Lc)
        lengths = (xc != 0).sum(axis=1).astype(f32)
        lenrep = np.ascontiguousarray(
            np.broadcast_to(lengths[None, :], (128, nl)))
        e = emb[xc]                                   # [nl, Lc, E]
        eT = np.ascontiguousarray(e.transpose(1, 2, 0))  # [Lc, E, nl]
        in_maps.append(dict(shared, eT=eT, lenrep=lenrep))
    return in_maps


def kernel(**inputs):
    nc = _cached_module(BL)
    in_maps = _host_prep(inputs, BL)
    res = bass_utils.run_bass_kernel_spmd(nc, in_maps,
                                          core_ids=list(range(NCORE)))
    nl = BL * S
    out = np.empty((B, S, OUT), np.float32)
    for c in range(NCORE):
        out[c * BL:(c + 1) * BL] = res.results[c]["y"].reshape(BL, S, OUT)
    return out
